# revision 31
# baseline (speedup 1.0000x reference)
"""NodeClustering (vq_codebook) Trainium2 kernel — bf16 rewrite.

Math (per batch element b, P=16384 points, C=256 channels, K=8 clusters):
  nodes = F_p @ proj_w.T + proj_b
  3 iterations of: sim = l2(nodes) @ l2(centers).T ; assign = argmax;
                   centers = segment_mean(nodes)
  weights = softmax(10 * l2(nodes) @ l2(centers).T)
  out = (weights@centers + F_p) @ refine_w.T + refine_b

Key algebraic restructuring so `nodes` is never materialized:
  nodes[p] . cn_k         = F_p[p] . G_k + h_k    (G_k = proj_w.T @ cn_k, h_k = proj_b . cn_k)
  segment_sum(nodes)      = (segment_sum(F_p)) @ proj_w.T + counts * proj_b
  out                     = F_p @ refine_w.T + weights @ (centers @ refine_w.T + refine_b)
                            (refine_b folds into Dm because softmax weights sum to 1)
  ||nodes_p||^2           = ||F_p@proj_w.T||^2 + F_p.(2 proj_w.T@proj_b) + ||proj_b||^2
                            (the cross term rides as a 257th rhs column of the proj matmul)

Perf structure: F_p is read from HBM exactly once and kept resident in SBUF
in bf16 in BOTH layouts (natural [p,c] for segment sums, transposed [c,p]
for everything contracting over channels), so iterations do no DMA. All
large matmuls run in bf16 (1 PE cycle/row vs 4 for fp32, single-pass
weight loads). Counts ride as a ones column of the natural-layout resident.

Sharding: pure data parallel, core i <- batch element i (B=8, 8 cores).
"""

import sys
import numpy as np

sys.path.insert(0, "/opt/trn_rl_repo")

import concourse.bass as bass
import concourse.bacc as bacc
import concourse.mybir as mybir
import concourse.tile as tile
from concourse._compat import get_trn_type
from concourse.bass_utils import axon_active
from concourse.masks import make_identity
from concourse.bass_utils import run_bass_kernel_spmd

P = 16384
C = 256
NK = 8
NUM_ITERS = 3
EPS = 1e-12
N_CORES = 8
NCHUNK = P // 128  # 128 chunks of 128 points
U_IN = 8   # chunks per input DMA (1 MiB)
U_OUT = 4  # chunks per output DMA (512 KiB)

F32 = mybir.dt.float32
BF16 = mybir.dt.bfloat16
AF = mybir.ActivationFunctionType


def build_bass(p=P):
    nchunk = p // 128
    idx = list(np.linspace(0, p - 1, NK).astype(np.int64))
    nc = bacc.Bacc(
        get_trn_type() or "TRN2",
        target_bir_lowering=False,
        debug=not axon_active(),
        num_devices=N_CORES,
    )

    fp = nc.dram_tensor("fp", [p, C], F32, kind="ExternalInput")
    pw = nc.dram_tensor("pw", [C, C], F32, kind="ExternalInput")
    pb = nc.dram_tensor("pb", [C], F32, kind="ExternalInput")
    rw = nc.dram_tensor("rw", [C, C], F32, kind="ExternalInput")
    rb = nc.dram_tensor("rb", [C], F32, kind="ExternalInput")
    out = nc.dram_tensor("out", [p, C], F32, kind="ExternalOutput")

    # chunk-major views: chunk n holds points n*128 .. n*128+127 on partitions
    fp_v = fp[:].rearrange("(n p) c -> p n c", p=128)
    out_v = out[:].rearrange("(n p) c -> p n c", p=128)

    with tile.TileContext(nc) as tc:
        with (
            tc.tile_pool(name="res", bufs=1) as res,        # persistent tiles
            tc.tile_pool(name="nat", bufs=3) as natp,       # streamed F_p chunks
            tc.tile_pool(name="outp", bufs=3) as outp,      # output staging
            tc.tile_pool(name="sml", bufs=6) as sml,        # per-chunk small tiles
            tc.tile_pool(name="scr", bufs=2) as scr,        # square scratch
            tc.tile_pool(name="it", bufs=2) as itp,         # per-iteration small tiles
            tc.tile_pool(name="ps_t", bufs=3, space="PSUM") as ps_t,    # transposes + sims
            tc.tile_pool(name="ps_n", bufs=3, space="PSUM") as ps_n,    # wide rows
            tc.tile_pool(name="ps_acc", bufs=1, space="PSUM") as ps_acc,  # accumulators + smalls
        ):
            # ---------------- phase 0: constants + weights ----------------
            ident = res.tile([128, 128], F32)
            make_identity(nc, ident)
            ident_b = res.tile([128, 128], BF16)
            make_identity(nc, ident_b)
            ones_row = res.tile([1, 128], F32)   # lhsT for broadcast matmuls
            nc.vector.memset(ones_row, 1.0)
            ones_rowB = res.tile([1, 128], BF16)  # bf16 lhsT for +h bias matmuls
            nc.vector.memset(ones_rowB, 1.0)

            pw_n = res.tile([128, 2, C], F32)    # proj_w rows (c' partition)
            nc.sync.dma_start(out=pw_n, in_=pw[:].rearrange("(h p) c -> p h c", p=128))
            rw_n = res.tile([128, 2, C], F32)
            nc.sync.dma_start(out=rw_n, in_=rw[:].rearrange("(h p) c -> p h c", p=128))
            pb_col = res.tile([128, 2], F32)     # proj_b as column halves
            nc.sync.dma_start(out=pb_col, in_=pb[:].rearrange("(h p) -> p h", p=128))
            pb_row = res.tile([1, C], F32)
            nc.sync.dma_start(out=pb_row, in_=pb[:].unsqueeze(0))
            rb_row = res.tile([1, C], F32)
            nc.sync.dma_start(out=rb_row, in_=rb[:].unsqueeze(0))

            # transposed weights: pwT[h] = proj_w.T rows h*128.. (c' partition, c free)
            pwT = res.tile([128, 2, C], F32)
            rwT = res.tile([128, 2, C], F32)
            for dst, src in ((pwT, pw_n), (rwT, rw_n)):
                for kh in range(2):      # source partition half (c')
                    for mh in range(2):  # source free half (c)
                        tp = ps_t.tile([128, 128], F32, tag="pst")
                        nc.tensor.transpose(tp, src[:, kh, mh * 128:(mh + 1) * 128], ident)
                        eng = nc.vector if (kh + mh) % 2 else nc.scalar
                        if eng is nc.vector:
                            nc.vector.tensor_copy(dst[:, mh, kh * 128:(kh + 1) * 128], tp)
                        else:
                            nc.scalar.activation(dst[:, mh, kh * 128:(kh + 1) * 128], tp,
                                                 AF.Copy)

            # bf16 fused rhs tensors
            # projrhsB[:, h, 0:256] = pwT half h; [:, :, 256] = 2*proj_w.T@proj_b
            # halves; [:, h, 257:265] = iteration-1 G half h (for the fused
            # first clustering pass)
            projrhsB = res.tile([128, 2, C + 1 + NK], BF16)
            # catB[:, h, 0:256] = rwT half h; [:, h, 256:264] = final-G half h (phase 5)
            catB = res.tile([128, 2, C + NK], BF16)
            for h in range(2):
                nc.vector.tensor_copy(projrhsB[:, h, 0:C], pwT[:, h])
                nc.scalar.activation(catB[:, h, 0:C], rwT[:, h], AF.Copy)
            # v2 = 2 * proj_w.T @ proj_b, split into column halves
            for mh in range(2):
                vp = ps_acc.tile([128, 1], F32, tag="ips")
                nc.tensor.matmul(vp, pw_n[:, 0, mh * 128:(mh + 1) * 128], pb_col[:, 0:1],
                                 start=True, stop=False)
                nc.tensor.matmul(vp, pw_n[:, 1, mh * 128:(mh + 1) * 128], pb_col[:, 1:2],
                                 start=False, stop=True)
                nc.scalar.activation(projrhsB[:, mh, C:C + 1], vp, AF.Copy, scale=2.0)
            # pbb = ||proj_b||^2 broadcast to a [128,1] column
            pbsq = scr.tile([1, C], F32, tag="sq")
            pbb1 = itp.tile([1, 1], F32, tag="pbb1")
            nc.scalar.activation(pbsq, pb_row, AF.Square, accum_out=pbb1)
            pbb_ps = ps_acc.tile([128, 1], F32, tag="ips")
            nc.tensor.matmul(pbb_ps, ones_row, pbb1)
            pbb_col = res.tile([128, 1], F32)
            nc.vector.tensor_copy(pbb_col, pbb_ps)

            # seed rows of F_p for initial centers, gathered straight from DRAM
            gatT = itp.tile([NK, C], F32, tag="gatT")
            for k, g in enumerate(idx):
                nc.sync.dma_start(out=gatT[k:k + 1, :], in_=fp[g:g + 1, :])

            # residents
            natB = res.tile([128, nchunk, C + 1], BF16)  # F_p natural + ones col
            nc.vector.memset(natB[:, :, C:C + 1], 1.0)
            fT0B = res.tile([128, p], BF16)   # F_p.T rows 0..127   (c partition)
            fT1B = res.tile([128, p], BF16)   # F_p.T rows 128..255
            SQ = res.tile([128, nchunk], F32)     # ||F_p@pwT||^2 per point
            CR = res.tile([128, nchunk], F32)     # cross term per point
            inv10 = res.tile([128, nchunk], F32)  # 10 / max(||nodes_p||, eps)

            def make_G(centers_sb, Gdst):
                """centers (8,C) -> l2norm -> bf16 G halves into Gdst(h) + hb (128,8) f32.

                Gdst: callable h -> AP([128, NK], bf16) destination for half h.
                """
                csq = itp.tile([NK, C], F32, tag="csq")
                cn2 = itp.tile([NK, 1], F32, tag="cn2")
                nc.scalar.activation(csq, centers_sb, AF.Square, accum_out=cn2)
                nc.scalar.sqrt(cn2, cn2)
                nc.vector.tensor_scalar(cn2, cn2, EPS, None, op0=mybir.AluOpType.max)
                rin = itp.tile([NK, 1], F32, tag="rin")
                nc.vector.reciprocal(rin, cn2)
                cn = itp.tile([NK, C], F32, tag="cn")
                nc.vector.tensor_scalar_mul(cn, centers_sb, rin)
                # cnT (c' partition, k free)
                cnT = itp.tile([128, 2, NK], F32, tag="cnT")
                for h in range(2):
                    tp = ps_acc.tile([128, NK], F32, tag="ips")
                    nc.tensor.transpose(tp, cn[:, h * 128:(h + 1) * 128], ident[0:NK, 0:NK])
                    nc.vector.tensor_copy(cnT[:, h], tp)
                # G[c,k] = sum_c' proj_w[c',c] cnT[c',k]
                for mh in range(2):
                    gp = ps_acc.tile([128, NK], F32, tag="ips")
                    nc.tensor.matmul(gp, pw_n[:, 0, mh * 128:(mh + 1) * 128], cnT[:, 0],
                                     start=True, stop=False)
                    nc.tensor.matmul(gp, pw_n[:, 1, mh * 128:(mh + 1) * 128], cnT[:, 1],
                                     start=False, stop=True)
                    nc.vector.tensor_copy(Gdst(mh), gp)
                # h = proj_b . cn_k (as a bf16 row for the PE +h bias matmul)
                hp = ps_acc.tile([1, NK], F32, tag="ips")
                nc.tensor.matmul(hp, pb_col[:, 0:1], cnT[:, 0], start=True, stop=False)
                nc.tensor.matmul(hp, pb_col[:, 1:2], cnT[:, 1], start=False, stop=True)
                hrowB = itp.tile([1, NK], BF16, tag="hrowB")
                nc.vector.tensor_copy(hrowB, hp)
                return hrowB

            def update_centers(S_ps):
                # fmean = S / max(counts, 1); centers = fmean @ proj_w.T + proj_b
                cnt = itp.tile([NK, 1], F32, tag="cntsb")
                nc.vector.tensor_scalar(cnt, S_ps[:, C:C + 1], 1.0, None,
                                        op0=mybir.AluOpType.max)
                nc.vector.reciprocal(cnt, cnt)
                fmean = itp.tile([NK, C], F32, tag="fmean")
                nc.vector.tensor_scalar_mul(fmean, S_ps[:, 0:C], cnt)
                fmT = itp.tile([128, 2, NK], F32, tag="fmT")
                for h in range(2):
                    tp = ps_acc.tile([128, NK], F32, tag="ips")
                    nc.tensor.transpose(tp, fmean[:, h * 128:(h + 1) * 128],
                                        ident[0:NK, 0:NK])
                    nc.vector.tensor_copy(fmT[:, h], tp)
                cp = ps_acc.tile([NK, C], F32, tag="ips")
                nc.tensor.matmul(cp, fmT[:, 0], pwT[:, 0], start=True, stop=False)
                nc.tensor.matmul(cp, fmT[:, 1], pwT[:, 1], start=False, stop=False)
                nc.tensor.matmul(cp, ones_row[:, 0:NK], pb_row, start=False, stop=True)
                centers = itp.tile([NK, C], F32, tag="centers")
                nc.vector.tensor_copy(centers, cp)
                return centers

            def argmax_onehot(sim_ps):
                # sim_ps already includes the +h bias (PE bias matmul)
                simsb = sml.tile([128, NK], F32, tag="simsb")
                nc.scalar.activation(simsb, sim_ps, AF.Copy)
                mx = sml.tile([128, NK], F32, tag="mx")
                nc.vector.max(mx, simsb)
                oh = sml.tile([128, NK], BF16, tag="oh")
                nc.vector.tensor_scalar(oh, simsb, mx[:, 0:1], None,
                                        op0=mybir.AluOpType.is_ge)
                return oh

            # initial centers from the DRAM-gathered seed rows
            gatc = itp.tile([128, 2, NK], F32, tag="gatc")
            for h in range(2):
                tp = ps_acc.tile([128, NK], F32, tag="ips")
                nc.tensor.transpose(tp, gatT[:, h * 128:(h + 1) * 128],
                                    ident[0:NK, 0:NK])
                nc.vector.tensor_copy(gatc[:, h], tp)
            c0 = ps_acc.tile([NK, C], F32, tag="ips")
            nc.tensor.matmul(c0, gatc[:, 0], pwT[:, 0], start=True, stop=False)
            nc.tensor.matmul(c0, gatc[:, 1], pwT[:, 1], start=False, stop=False)
            nc.tensor.matmul(c0, ones_row[:, 0:NK], pb_row, start=False, stop=True)
            centers = itp.tile([NK, C], F32, tag="centers")
            nc.vector.tensor_copy(centers, c0)
            # iteration-1 G rides as columns 257:265 of the phase-1 proj rhs
            hrow1B = make_G(centers, lambda h: projrhsB[:, h, C + 1:C + 1 + NK])

            # ---- phase 1: load + cast + transpose + norms + fused iteration 1 ----
            S_ps = ps_acc.tile([NK, C + 1], F32, tag="S")
            for gi in range(nchunk // U_IN):
                nt = natp.tile([128, U_IN, C], F32, tag="nat")
                nc.sync.dma_start(out=nt, in_=fp_v[:, gi * U_IN:(gi + 1) * U_IN, :])
                for j in range(U_IN):
                    ci = gi * U_IN + j
                    sl = slice(ci * 128, (ci + 1) * 128)
                    nc.vector.tensor_copy(natB[:, ci, 0:C], nt[:, j, :])
                    t0 = ps_t.tile([128, 128], BF16, tag="pst")
                    t1 = ps_t.tile([128, 128], BF16, tag="pst")
                    nc.tensor.transpose(t0, natB[:, ci, 0:128], ident_b)
                    nc.tensor.transpose(t1, natB[:, ci, 128:256], ident_b)
                    nc.scalar.activation(fT0B[:, sl], t0, AF.Copy)
                    nc.scalar.activation(fT1B[:, sl], t1, AF.Copy)
                    # y = F_p @ [pwT | 2 pwT@pb | G1]: norms + iteration-1 sim (+h)
                    y = ps_n.tile([128, C + 1 + NK], F32, tag="big")
                    nc.tensor.matmul(y, fT0B[:, sl], projrhsB[:, 0], start=True, stop=False)
                    nc.tensor.matmul(y[:, C + 1:C + 1 + NK], ones_rowB, hrow1B,
                                     start=False, stop=False, skip_group_check=True)
                    nc.tensor.matmul(y, fT1B[:, sl], projrhsB[:, 1], start=False, stop=True)
                    # SQ[ci] = ||F_p@pwT||^2 ; CR[ci] = cross term
                    sq = scr.tile([128, C], F32, tag="sq")
                    nc.scalar.activation(sq, y[:, 0:C], AF.Square,
                                         accum_out=SQ[:, ci:ci + 1])
                    nc.vector.tensor_copy(CR[:, ci:ci + 1], y[:, C:C + 1])
                    oh = argmax_onehot(y[:, C + 1:C + 1 + NK])
                    nc.tensor.matmul(S_ps, oh, natB[:, ci, :],
                                     start=(ci == 0), stop=(ci == nchunk - 1))
            # finalize inv10 = 10 / max(sqrt(SQ + CR + pbb), eps) with wide ops
            nc.vector.tensor_add(SQ, SQ, CR)
            nrm = scr.tile([128, nchunk], F32, tag="nrm")
            nc.scalar.activation(nrm, SQ, AF.Sqrt, bias=pbb_col)
            nc.vector.tensor_scalar(nrm, nrm, EPS, 0.1,
                                    op0=mybir.AluOpType.max,
                                    op1=mybir.AluOpType.mult)
            nc.vector.reciprocal(inv10, nrm)
            centers = update_centers(S_ps)

            # ---------------- phases 2-4: remaining clustering iterations ----------------
            for it in range(NUM_ITERS - 1):
                GB = itp.tile([128, 2, NK], BF16, tag="GB")
                hrowB = make_G(centers, lambda h: GB[:, h])
                S_ps = ps_acc.tile([NK, C + 1], F32, tag="S")
                for ci in range(nchunk):
                    sl = slice(ci * 128, (ci + 1) * 128)
                    sim = ps_t.tile([128, NK], F32, tag="pst")
                    nc.tensor.matmul(sim, fT0B[:, sl], GB[:, 0], start=True, stop=False)
                    nc.tensor.matmul(sim, fT1B[:, sl], GB[:, 1], start=False, stop=False)
                    nc.tensor.matmul(sim, ones_rowB, hrowB, start=False, stop=True)
                    oh = argmax_onehot(sim)
                    nc.tensor.matmul(S_ps, oh, natB[:, ci, :],
                                     start=(ci == 0), stop=(ci == nchunk - 1))
                centers = update_centers(S_ps)

            # ---------------- phase 5: final weights + refine ----------------
            hrowFB = make_G(centers, lambda h: catB[:, h, C:C + NK])
            # Dm = centers @ refine_w.T + refine_b (refine_b folds in: weights sum to 1)
            cT = itp.tile([128, 2, NK], F32, tag="cT")
            for h in range(2):
                tp = ps_acc.tile([128, NK], F32, tag="ips")
                nc.tensor.transpose(tp, centers[:, h * 128:(h + 1) * 128], ident[0:NK, 0:NK])
                nc.vector.tensor_copy(cT[:, h], tp)
            dm_ps = ps_acc.tile([NK, C], F32, tag="ips")
            nc.tensor.matmul(dm_ps, cT[:, 0], rwT[:, 0], start=True, stop=False)
            nc.tensor.matmul(dm_ps, cT[:, 1], rwT[:, 1], start=False, stop=False)
            nc.tensor.matmul(dm_ps, ones_row[:, 0:NK], rb_row, start=False, stop=True)
            DmB = itp.tile([NK, C], BF16, tag="DmB")
            nc.vector.tensor_copy(DmB, dm_ps)

            for gi in range(nchunk // U_OUT):
                ot = outp.tile([128, U_OUT, C], F32, tag="ot")
                for j in range(U_OUT):
                    ci = gi * U_OUT + j
                    sl = slice(ci * 128, (ci + 1) * 128)
                    op_ = ps_n.tile([128, C + NK], F32, tag="big")
                    nc.tensor.matmul(op_, fT0B[:, sl], catB[:, 0], start=True, stop=False)
                    nc.tensor.matmul(op_[:, C:C + NK], ones_rowB, hrowFB,
                                     start=False, stop=False, skip_group_check=True)
                    nc.tensor.matmul(op_, fT1B[:, sl], catB[:, 1], start=False, stop=True)
                    esim = sml.tile([128, NK], F32, tag="esim")
                    den = sml.tile([128, 1], F32, tag="den")
                    nc.scalar.activation(esim, op_[:, C:C + NK], AF.Exp,
                                         scale=inv10[:, ci:ci + 1], accum_out=den)
                    nc.vector.reciprocal(den, den)
                    wgt = sml.tile([128, NK], BF16, tag="wgt")
                    nc.vector.tensor_scalar_mul(wgt, esim, den)
                    wT_ps = ps_t.tile([NK, 128], BF16, tag="pst")
                    nc.tensor.transpose(wT_ps, wgt, ident_b)
                    wT = sml.tile([NK, 128], BF16, tag="wTsb")
                    nc.scalar.activation(wT, wT_ps, AF.Copy)
                    nc.tensor.matmul(op_[:, 0:C], wT, DmB,
                                     start=False, stop=True, skip_group_check=True)
                    nc.scalar.activation(ot[:, j, 0:128], op_[:, 0:128], AF.Copy)
                    nc.vector.tensor_copy(ot[:, j, 128:256], op_[:, 128:256])
                nc.sync.dma_start(out=out_v[:, gi * U_OUT:(gi + 1) * U_OUT, :], in_=ot)

    nc.compile()
    return nc


_NC = None
TRACE = False
TRACE_DIR = None
LAST_EXEC_NS = None


def kernel(F_p, proj_w, proj_b, refine_w, refine_b):
    global _NC, LAST_EXEC_NS
    if _NC is None:
        _NC = build_bass()
    F_p = np.ascontiguousarray(F_p, dtype=np.float32)
    shared = {
        "pw": np.ascontiguousarray(proj_w, dtype=np.float32),
        "pb": np.ascontiguousarray(proj_b, dtype=np.float32),
        "rw": np.ascontiguousarray(refine_w, dtype=np.float32),
        "rb": np.ascontiguousarray(refine_b, dtype=np.float32),
    }
    in_maps = [{"fp": F_p[i], **shared} for i in range(N_CORES)]
    res = run_bass_kernel_spmd(_NC, in_maps, list(range(N_CORES)), trace=TRACE,
                               tmpdir=TRACE_DIR)
    LAST_EXEC_NS = res.exec_time_ns
    return np.stack([res.results[i]["out"] for i in range(N_CORES)], axis=0)
